# revision 3
# baseline (speedup 1.0000x reference)
"""Trainium2 Bass kernel v2 for 2-layer LSTM (B=64, T=512, D=64, H=512, out=32).

Data-parallel over batch across 8 cores (B_local=8), weights replicated.

Key design vs v1:
  - Sigmoid-ONLY activations: tanh(x) = 2*sigmoid(2x) - 1, with the 2x folded
    into host-side weights for the g-gate and the affine fixup into fused DVE
    ops.  No sigmoid<->tanh ACT-table reloads (1.28us each!).
  - W_hh in fp8 e3m4 (x128 host scale; g-gate rows x256) -> FWL weight loads
    run 2x faster than bf16.  Compensated via sigmoid's scale=1/128 input
    scaling; h stays unscaled.
  - Merged gate layout: one [128,128] PSUM tile per step, col = m*8+b with
    m = gate*4+k (gate-major -> all ACT/DVE slices contiguous).
  - xg contribution injected into PSUM via an identity-weight matmul
    (no DVE add on the critical chain).
  - L0/L1 wavefront: layer-1 recurrence (lagged LAG steps) interleaves with
    layer-0 on the same core, so each layer's nonlinear tail hides under the
    other layer's matmul phase.  xg1 is computed block-wise (16 steps) into an
    SBUF ring just in time.
"""

import numpy as np
import ml_dtypes

import concourse.bass as bass
import concourse.mybir as mybir
import concourse.tile as tile
from concourse.bass_utils import run_bass_kernel_spmd

# ---------------------------------------------------------------------------
# walrus workaround: split the final TileContext drain (multi-sem-wait CTRL
# instruction) into one drain per proc; installed walrus caps waits at 1.
from concourse.vector_clock import ScopedClock, VectorClock


def _drain_and_barrier_split(self, tick_clock, wait_clock):
    gc = tick_clock.global_clock
    n = len(gc)
    emitted = 0
    for p in range(n):
        if gc[p] > 0:
            v = [0] * n
            v[p] = gc[p]
            d = self.nc.sync.drain()
            wait_clock.add_sem_waits(d.ins, ScopedClock({None: VectorClock(v)}))
            emitted += 1
    if emitted == 0:
        self.nc.sync.drain()
    self.nc.all_engine_barrier()
    assert self.sems is not None
    popped = self.nc._tile_sem_poison_stack.pop()
    assert popped is self._sem_poison
    self.nc.clear_and_free_semaphores(list(self.sems.allocated().values()))
    self.nc.all_engine_barrier()


_orig_drain_and_barrier = tile.TileContext._drain_and_barrier

import bass_rust

_wsplit_ctr = [0]


def _split_multi_waits(nc):
    """walrus also caps waits at 1 on regular instructions: move extra waits
    onto same-engine NoOps inserted immediately before."""
    for fn in nc.m.functions:
        for blk in fn.blocks:
            insts = blk.instructions
            i = 0
            while i < len(insts):
                inst = insts[i]
                si = inst.sync_info
                if si is not None and len(si.on_wait) > 1:
                    waits = list(si.on_wait)
                    si.on_wait = [waits[-1]]
                    for w in waits[:-1]:
                        _wsplit_ctr[0] += 1
                        no = mybir.InstNoOp(
                            name=f"wsplit_{_wsplit_ctr[0]}", ins=[], outs=[])
                        no.engine = inst.engine
                        no.sync_info = bass_rust.SyncInfo(
                            on_wait=[w], on_update=[])
                        insts.insert(i, no)
                        i += 1
                i += 1
# ---------------------------------------------------------------------------

F32 = mybir.dt.float32
BF16 = mybir.dt.bfloat16
FP8 = mybir.dt.float8e3
AF = mybir.ActivationFunctionType
ALU = mybir.AluOpType

B, D_IN, H, D_OUT = 64, 64, 512, 32
G = 4 * H
BL = 8             # batch per core
NCORES = 8
KT = H // 128      # 4 hidden chunks
MT = G // 128      # 16 gate tiles
SCALE = 128.0      # fp8 weight scale (g-gate rows get 2*SCALE)
BLK = 16           # xg1 block size (steps)
LAG = 17           # L1 lags L0 by LAG steps


def build_kernel(T, hw=True):
    tile.TileContext._drain_and_barrier = (
        _drain_and_barrier_split if hw else _orig_drain_and_barrier)
    blk = min(BLK, T)
    assert T % blk == 0
    NTOK = T * BL
    nch = max(1, NTOK // 512)   # phase-B token chunks of 512
    CH = NTOK // nch

    nc = bass.Bass()

    xT_d = nc.declare_dram_parameter("xT", [D_IN, NTOK], BF16, isOutput=False)
    wih0_d = nc.declare_dram_parameter("Wih0T", [D_IN, G], BF16, isOutput=False)
    whh0_d = nc.declare_dram_parameter("Whh0T", [H, G], FP8, isOutput=False)
    wih1_d = nc.declare_dram_parameter("Wih1T", [H, G], BF16, isOutput=False)
    whh1_d = nc.declare_dram_parameter("Whh1T", [H, G], FP8, isOutput=False)
    wout_d = nc.declare_dram_parameter("WoutT", [H, D_OUT], BF16, isOutput=False)
    b0_d = nc.declare_dram_parameter("b0", [1, G], BF16, isOutput=False)
    b1_d = nc.declare_dram_parameter("b1", [1, G], BF16, isOutput=False)
    bout_d = nc.declare_dram_parameter("bout", [D_OUT, 1], F32, isOutput=False)
    ident_d = nc.declare_dram_parameter("ident", [128, 128], FP8, isOutput=False)
    y_d = nc.declare_dram_parameter("yT", [D_OUT, BL], F32, isOutput=True)

    # xg0 staged m-major: [m, p, t*8+b]
    xg0_d = nc.dram_tensor("xg0", [MT, 128, NTOK], BF16)

    with tile.TileContext(nc) as tc:
        with (
            tc.tile_pool(name="w", bufs=1) as wpool,
            tc.tile_pool(name="xg", bufs=6) as xg_pool,
            tc.tile_pool(name="cp", bufs=3) as cp_pool,
            tc.tile_pool(name="s", bufs=3) as s_pool,
            tc.tile_pool(name="sc", bufs=3) as sc_pool,
            tc.tile_pool(name="tmp", bufs=4) as tmp_pool,
            tc.tile_pool(name="ps0", bufs=2, space="PSUM") as ps0_pool,
            tc.tile_pool(name="ps1", bufs=2, space="PSUM") as ps1_pool,
            tc.tile_pool(name="pspre", bufs=2, space="PSUM") as pspre_pool,
            tc.tile_pool(name="psB", bufs=2, space="PSUM") as psB_pool,
        ):
            # ---- load weights / persistent state ----
            xT = wpool.tile([D_IN, NTOK], BF16, name="xT", tag="xT")
            nc.sync.dma_start(xT[:], xT_d[:])
            wih0 = wpool.tile([D_IN, G], BF16, name="wih0", tag="wih0")
            nc.sync.dma_start(wih0[:], wih0_d[:])
            whh0 = [wpool.tile([128, G], FP8, name=f"whh0_{k}", tag=f"whh0_{k}")
                    for k in range(KT)]
            wih1 = [wpool.tile([128, G], BF16, name=f"wih1_{k}", tag=f"wih1_{k}")
                    for k in range(KT)]
            whh1 = [wpool.tile([128, G], FP8, name=f"whh1_{k}", tag=f"whh1_{k}")
                    for k in range(KT)]
            wout = [wpool.tile([128, D_OUT], BF16, name=f"wout_{k}", tag=f"wout_{k}")
                    for k in range(KT)]
            for k in range(KT):
                sl = slice(128 * k, 128 * (k + 1))
                nc.sync.dma_start(whh0[k][:], whh0_d[sl, :])
                nc.sync.dma_start(wih1[k][:], wih1_d[sl, :])
                nc.sync.dma_start(whh1[k][:], whh1_d[sl, :])
                nc.sync.dma_start(wout[k][:], wout_d[sl, :])
            b0t = wpool.tile([1, G], BF16, name="b0", tag="b0")
            nc.sync.dma_start(b0t[:], b0_d[:])
            b1t = wpool.tile([1, G], BF16, name="b1", tag="b1")
            nc.sync.dma_start(b1t[:], b1_d[:])
            boutt = wpool.tile([D_OUT, 1], F32, name="bout", tag="bout")
            nc.sync.dma_start(boutt[:], bout_d[:])
            identt = wpool.tile([128, 128], FP8, name="ident", tag="ident")
            nc.sync.dma_start(identt[:], ident_d[:])

            ones = wpool.tile([1, CH], BF16, name="ones", tag="ones")
            nc.gpsimd.memset(ones[:], 1.0)
            z8 = wpool.tile([128, BL], BF16, name="z8", tag="z8")
            nc.gpsimd.memset(z8[:], 0.0)

            h1seq = wpool.tile([128, T * 32], BF16, name="h1seq", tag="h1seq")
            hL1 = wpool.tile([128, 64], BF16, name="hL1", tag="hL1")
            c0t = wpool.tile([128, 32], F32, name="c0", tag="c0")
            nc.gpsimd.memset(c0t[:], 0.0)
            c1t = wpool.tile([128, 32], F32, name="c1", tag="c1")
            nc.gpsimd.memset(c1t[:], 0.0)
            xg1ring = wpool.tile([128, 2 * blk * 128], BF16, name="xg1r",
                                 tag="xg1r")

            # ---- phase B: xg0 = (Wih0 @ x + b0), staged m-major in DRAM ----
            for m in range(MT):
                msl = slice(m * 128, (m + 1) * 128)
                for ch in range(nch):
                    csl = slice(ch * CH, (ch + 1) * CH)
                    ps = psB_pool.tile([128, CH], F32, name="psB", tag="psB")
                    nc.tensor.matmul(ps[:], wih0[:, msl], xT[:, csl],
                                     start=True, stop=False)
                    nc.tensor.matmul(ps[:], b0t[0:1, msl], ones[0:1, :],
                                     start=False, stop=True)
                    cp = cp_pool.tile([128, CH], BF16, name="cp", tag="cp")
                    if (m * nch + ch) % 2 == 0:
                        nc.vector.tensor_copy(cp[:], ps[:])
                    else:
                        nc.scalar.copy(cp[:], ps[:])
                    nc.sync.dma_start(xg0_d[m, :, csl], cp[:])

            # ---- wavefront: L0 step ss | xg1 block | L1 step ss-LAG ----
            h1seq_r = None

            def emit_step(layer, t, psum_pool, whh, xg_ap, c_tile, h_dst,
                          h_src):
                psum = psum_pool.tile([128, 128], F32, name=f"g{layer}",
                                      tag=f"g{layer}")
                nc.tensor.matmul(psum[:], identt[:], xg_ap,
                                 start=True, stop=False)
                for kk in range(KT):
                    rhs = h_src(kk)
                    for m in range(MT):
                        nc.tensor.matmul(
                            psum[:, m * 8:(m + 1) * 8],
                            whh[kk][:, m * 128:(m + 1) * 128],
                            rhs,
                            start=False,
                            stop=(kk == KT - 1 and m == MT - 1),
                        )
                s = s_pool.tile([128, 128], F32, name=f"s{layer}",
                                tag=f"s{layer}")
                nc.scalar.activation(s[:], psum[:], AF.Sigmoid,
                                     scale=1.0 / SCALE)
                wg = tmp_pool.tile([128, 32], F32, name="wg", tag="wg")
                nc.vector.tensor_scalar(wg[:], s[:, 96:128], 2.0, -1.0,
                                        ALU.mult, ALU.add)
                t1 = tmp_pool.tile([128, 32], F32, name="t1", tag="t1")
                nc.vector.tensor_mul(t1[:], s[:, 0:32], wg[:])
                m2 = tmp_pool.tile([128, 32], F32, name="m2", tag="m2")
                nc.gpsimd.tensor_mul(m2[:], s[:, 32:64], c_tile[:])
                nc.vector.tensor_add(c_tile[:], t1[:], m2[:])
                s_c = sc_pool.tile([128, 32], F32, name=f"sc{layer}",
                                   tag=f"sc{layer}")
                nc.scalar.activation(s_c[:], c_tile[:], AF.Sigmoid, scale=2.0)
                wc = tmp_pool.tile([128, 32], F32, name="wc", tag="wc")
                nc.vector.tensor_scalar(wc[:], s_c[:], 2.0, -1.0,
                                        ALU.mult, ALU.add)
                nc.vector.tensor_mul(h_dst, s[:, 64:96], wc[:])

            for ss in range(T + LAG):
                if ss < T:
                    # L0 step ss
                    xg_sb = xg_pool.tile([128, 128], BF16, name="xg", tag="xg")
                    nc.sync.dma_start(
                        xg_sb[:].rearrange("p (m b) -> p m b", b=BL),
                        xg0_d[:, :, ss * BL:(ss + 1) * BL].rearrange(
                            "m p b -> p m b"),
                    )
                    emit_step(
                        0, ss, ps0_pool, whh0, xg_sb[:], c0t,
                        h1seq[:, ss * 32:(ss + 1) * 32],
                        lambda kk, _t=ss: (z8[:] if _t == 0 else
                                           h1seq[:, (_t - 1) * 32 + kk * 8:
                                                 (_t - 1) * 32 + kk * 8 + 8]),
                    )
                    if ss % blk == blk - 1:
                        # xg1 GEMM for block Bk -> SBUF ring
                        Bk = ss // blk
                        slot = Bk % 2
                        hblk = h1seq[:, Bk * blk * 32:(Bk + 1) * blk * 32]
                        hblk_r = hblk.rearrange("p (t k b) -> p k t b",
                                                k=KT, b=BL)
                        ring_r = xg1ring[:].rearrange(
                            "p (s t c) -> p s t c", s=2, t=blk)
                        for m in range(MT):
                            msl = slice(m * 128, (m + 1) * 128)
                            ps = pspre_pool.tile([128, blk * BL], F32,
                                                 name="pre", tag="pre")
                            for kk in range(KT):
                                nc.tensor.matmul(
                                    ps[:], wih1[kk][:, msl], hblk_r[:, kk],
                                    start=(kk == 0), stop=False)
                            nc.tensor.matmul(
                                ps[:], b1t[0:1, msl], ones[0:1, 0:blk * BL],
                                start=False, stop=True)
                            nc.vector.tensor_copy(
                                ring_r[:, slot, :, m * 8:(m + 1) * 8],
                                ps[:].rearrange("p (t b) -> p t b", b=BL))
                if ss >= LAG:
                    t1s = ss - LAG
                    slot = (t1s // blk) % 2
                    base = slot * blk * 128 + (t1s % blk) * 128
                    xg_ap = xg1ring[:, base:base + 128]
                    emit_step(
                        1, t1s, ps1_pool, whh1, xg_ap, c1t,
                        hL1[:, (t1s % 2) * 32:(t1s % 2) * 32 + 32],
                        lambda kk, _t=t1s: (
                            z8[:] if _t == 0 else
                            hL1[:, ((_t - 1) % 2) * 32 + kk * 8:
                                ((_t - 1) % 2) * 32 + kk * 8 + 8]),
                    )

            # ---- y = Wout @ h_last + bout ----
            ps_y = ps0_pool.tile([D_OUT, BL], F32, name="g0", tag="g0")
            last = (T - 1) % 2
            for kk in range(KT):
                nc.tensor.matmul(
                    ps_y[:], wout[kk][:],
                    hL1[:, last * 32 + kk * 8:last * 32 + kk * 8 + 8],
                    start=(kk == 0), stop=(kk == KT - 1),
                )
            y_sb = sc_pool.tile([D_OUT, BL], F32, name="y_sb", tag="y_sb")
            nc.scalar.activation(y_sb[:], ps_y[:], AF.Identity,
                                 bias=boutt[:, 0:1])
            nc.sync.dma_start(y_d[:], y_sb[:])

    if hw:
        _split_multi_waits(nc)
    return nc


_NC_CACHE = {}


def _get_nc(T, hw=True):
    key = (T, hw)
    if key not in _NC_CACHE:
        _NC_CACHE[key] = build_kernel(T, hw=hw)
    return _NC_CACHE[key]


GATE_PERM = [0, 1, 3, 2]  # [i, f, o, g]


def _gperm(W):
    return np.ascontiguousarray(
        W.reshape(4, H, *W.shape[1:])[GATE_PERM].reshape(W.shape))


# row scale: SCALE for i,f,o rows; 2*SCALE for g rows (tanh = 2*sig(2x)-1)
def _row_scale():
    s = np.full((G, 1), SCALE, np.float32)
    s[3 * H:] = 2.0 * SCALE   # after GATE_PERM, g-gate occupies rows 3H:4H
    return s


def _prep_inputs(x, W_ih0, W_hh0, b_ih0, b_hh0, W_ih1, W_hh1, b_ih1, b_hh1,
                 W_out, b_out):
    bf = ml_dtypes.bfloat16
    f8 = ml_dtypes.float8_e3m4
    T = x.shape[1]
    rs = _row_scale()
    shared = {
        "Wih0T": np.ascontiguousarray((_gperm(W_ih0) * rs).T).astype(bf),
        "Whh0T": np.ascontiguousarray((_gperm(W_hh0) * rs).T).astype(f8),
        "Wih1T": np.ascontiguousarray((_gperm(W_ih1) * rs).T).astype(bf),
        "Whh1T": np.ascontiguousarray((_gperm(W_hh1) * rs).T).astype(f8),
        "WoutT": np.ascontiguousarray(W_out.T).astype(bf),
        "b0": (_gperm((b_ih0 + b_hh0).reshape(G, 1)) * rs).reshape(1, G).astype(bf),
        "b1": (_gperm((b_ih1 + b_hh1).reshape(G, 1)) * rs).reshape(1, G).astype(bf),
        "bout": b_out.reshape(D_OUT, 1).astype(np.float32),
        "ident": np.eye(128, dtype=np.float32).astype(f8),
    }
    in_maps = []
    for c in range(NCORES):
        xc = x[c * BL:(c + 1) * BL]            # [8, T, 64]
        xT = np.ascontiguousarray(xc.transpose(2, 1, 0).reshape(D_IN, T * BL))
        in_maps.append({"xT": xT.astype(bf), **shared})
    return in_maps


def kernel(x, W_ih0, W_hh0, b_ih0, b_hh0, W_ih1, W_hh1, b_ih1, b_hh1,
           W_out, b_out):
    T = x.shape[1]
    nc = _get_nc(T)
    in_maps = _prep_inputs(x, W_ih0, W_hh0, b_ih0, b_hh0, W_ih1, W_hh1,
                           b_ih1, b_hh1, W_out, b_out)
    res = run_bass_kernel_spmd(nc, in_maps, core_ids=list(range(NCORES)))
    out = np.concatenate(
        [res.results[c]["yT"].T for c in range(NCORES)], axis=0)
    return np.ascontiguousarray(out.astype(np.float32))
